# revision 1
# baseline (speedup 1.0000x reference)
"""DeepClusterLoss on 8 Trainium2 NeuronCores (Bass/Tile).

reference:
    recon_loss   = sum((recon_x - x)**2)
    cluster_loss = sum((x - centers[assign])**2)
    total        = recon_loss + cluster_loss          (ALPHA = BETA = 1)

Device strategy (data-parallel over N, per the sharding hint):
  - Inputs are streamed in bf16 (host-side cast, exact-to-tolerance: all
    outputs are ~1e8-magnitude sums of ~1e0 terms; the bf16 rounding noise
    averages to ~1e-6 relative).  This halves HBM traffic and unlocks the
    fast PE/DVE paths (1 cycle/row matmuls, single-pass LDWEIGHTS, 2x DVE).
  - Each sample is stored as 65 bf16s: [x_i (64) | flag], flag = 1.0 for
    real samples, 0.0 for padding.  recon_x rows carry the same flag, so
    (r - x) has an exact 0 in the flag column.
  - recon part: DVE computes d = r - x (bf16), ACT computes Square(d) with
    accum_out -> fp32 per-partition partials.  ACT Square(x) likewise (the
    flag column adds +1 per real sample; the host subtracts N afterwards).
  - cluster part avoids the gather:
        cluster = sum|x|^2 - 2*sum_k <S_k, C_k> + sum_k n_k*|C_k|^2
    S_k (segment sums) and n_k (counts) come from ONE matmul per
    128-sample slot: a one-hot [128, K] bf16 (tensor_scalar is_equal
    against an iota row; built on DVE and GpSimd in parallel) contracted
    with the augmented x-slot [128, 65] -> PSUM [K, 65] fp32, where column
    64 (the flag) accumulates exactly n_k.
  - Host combines the tiny per-core fp32 outputs in float64.

Padding uses assignment class K (=100): its one-hot row is all zeros, so
padded samples vanish from S and the counts.
"""

import sys
from contextlib import ExitStack

import numpy as np

for _p in ("/opt/trn_rl_repo", "/opt/pypackages"):
    if _p not in sys.path:
        sys.path.append(_p)

import ml_dtypes
import concourse.tile as tile
from concourse import bacc, mybir
from concourse.bass_utils import run_bass_kernel_spmd

N, D, K = 1_000_000, 64, 100
ALPHA, BETA = 1.0, 1.0
N_CORES = 8
N_PER_CORE = N // N_CORES  # 125000
P = 128                    # SBUF partitions
DA = D + 1                 # augmented sample width (x | flag)
SLOTS = 32                 # sample-slots per partition per tile
FREE = DA * SLOTS          # 2080 bf16 per partition per half-tile
SPT = P * SLOTS            # samples per tile = 4096
NTILES = -(-N_PER_CORE // SPT)  # 31
PADDED = NTILES * SPT      # 126976
PAD_CLASS = float(K)       # out-of-range class: one-hot row is all zeros
GP_FRAC = 3                # j % GP_FRAC == 0 -> one-hot built on GpSimd

_bf16 = mybir.dt.bfloat16
_f32 = mybir.dt.float32
BF16 = ml_dtypes.bfloat16


def build_nc(ntiles: int = NTILES):
    """Build + compile the per-core Bass program (same program on all cores)."""
    nc = bacc.Bacc()
    # x-aug and r-aug interleaved per tile: xr[t, p, 0:FREE] = x-aug,
    # xr[t, p, FREE:2*FREE] = r-aug  (one DMA per tile)
    xr_d = nc.dram_tensor("xr", [ntiles, P, 2 * FREE], _bf16, kind="ExternalInput")
    # host-precomputed one-hot rows, slot-major per tile: oh[t, p, j*K + k]
    # = 1.0 iff sample (t*SPT + p*SLOTS + j) has assignment k (pad rows are
    # all-zero).  Streaming these costs ~25 MB/core but removes every
    # per-slot DVE op from the kernel, leaving it DMA-bound.
    oh_d = nc.dram_tensor("oh", [ntiles, P, SLOTS * K], _bf16, kind="ExternalInput")
    s_out = nc.dram_tensor("s_out", [K, DA], _f32, kind="ExternalOutput")
    part_out = nc.dram_tensor("partials", [P, 2 * ntiles], _f32, kind="ExternalOutput")

    with ExitStack() as ctx:
        tc = ctx.enter_context(tile.TileContext(nc))
        const_pool = ctx.enter_context(tc.tile_pool(name="const", bufs=1))
        xin = ctx.enter_context(tc.tile_pool(name="xin", bufs=8))
        scratch = ctx.enter_context(tc.tile_pool(name="scratch", bufs=2))
        ohp = ctx.enter_context(tc.tile_pool(name="ohp", bufs=5))
        psum = ctx.enter_context(tc.tile_pool(name="psum", bufs=1, space="PSUM"))

        partials_sb = const_pool.tile([P, 2 * ntiles], _f32)

        s_psum = psum.tile([K, DA], _f32)

        for t in range(ntiles):
            xr_t = xin.tile([P, 2 * FREE], _bf16)
            nc.sync.dma_start(xr_t[:], xr_d[t, :, :])
            x_t = xr_t[:, 0:FREE]
            r_t = xr_t[:, FREE : 2 * FREE]

            d_t = scratch.tile([P, FREE], _bf16, tag="d")
            nc.vector.tensor_sub(d_t[:], r_t, x_t)
            sq_t = scratch.tile([P, FREE], _bf16, tag="sq")
            nc.scalar.activation(
                sq_t[:], d_t[:], mybir.ActivationFunctionType.Square,
                accum_out=partials_sb[:, t : t + 1],
            )
            sq2_t = scratch.tile([P, FREE], _bf16, tag="sq")
            nc.scalar.activation(
                sq2_t[:], x_t, mybir.ActivationFunctionType.Square,
                accum_out=partials_sb[:, ntiles + t : ntiles + t + 1],
            )

            oh_bf = ohp.tile([P, SLOTS * K], _bf16, tag="ohb")
            # issue on the ACT HW-DGE ring so the xr stream (SP ring) and the
            # one-hot stream generate descriptors in parallel
            nc.scalar.dma_start(oh_bf[:], oh_d[t, :, :])
            for j in range(SLOTS):
                nc.tensor.matmul(
                    s_psum[:],
                    oh_bf[:, j * K : (j + 1) * K],
                    x_t[:, j * DA : (j + 1) * DA],
                    start=(t == 0 and j == 0),
                    stop=(t == ntiles - 1 and j == SLOTS - 1),
                )

        s_sb = const_pool.tile([K, DA], _f32)
        nc.vector.tensor_copy(s_sb[:], s_psum[:])
        nc.sync.dma_start(s_out[:, :], s_sb[:])
        nc.sync.dma_start(part_out[:, :], partials_sb[:])

    nc.compile()
    return nc


def host_prepare(recon_x, x, cluster_assignments, ntiles: int = NTILES,
                 n_cores: int = N_CORES):
    """Shard + pad + cast + lay out the inputs for each core."""
    n_per_core = x.shape[0] // n_cores
    padded = ntiles * SPT
    x_np = np.asarray(x, dtype=np.float32).reshape(n_cores, n_per_core, D)
    r_np = np.asarray(recon_x, dtype=np.float32).reshape(n_cores, n_per_core, D)
    a_np = np.asarray(cluster_assignments).reshape(n_cores, n_per_core)

    xr = np.zeros((n_cores, ntiles, P, 2 * FREE), BF16)
    xa = np.zeros((n_cores, padded, DA), BF16)
    xa[:, :n_per_core, :D] = x_np.astype(BF16)
    xa[:, :n_per_core, D] = 1.0
    xr[:, :, :, 0:FREE] = xa.reshape(n_cores, ntiles, P, FREE)
    xa[:, :n_per_core, :D] = r_np.astype(BF16)   # reuse buffer for r-aug
    xr[:, :, :, FREE:] = xa.reshape(n_cores, ntiles, P, FREE)

    in_maps = []
    for c in range(n_cores):
        oh = np.zeros((padded, K), BF16)
        oh[np.arange(n_per_core), a_np[c].astype(np.int64)] = 1.0
        in_maps.append(
            {
                "xr": xr[c],
                "oh": oh.reshape(ntiles, P, SLOTS * K),
            }
        )
    return in_maps


def host_combine(results, cluster_centers, ntiles: int = NTILES,
                 n_real: int = N):
    """Reduce per-core outputs into (total, recon, cluster) in float64."""
    S = np.zeros((K, DA), np.float64)
    recon = 0.0
    xsq = 0.0
    for rd in results:
        S += rd["s_out"].astype(np.float64)
        pr = rd["partials"].astype(np.float64)
        recon += pr[:, :ntiles].sum()
        xsq += pr[:, ntiles:].sum()
    xsq -= n_real  # flag column contributes 1 per real sample
    cnt = S[:, D]
    C = np.asarray(cluster_centers, dtype=np.float64)
    cross = float((S[:, :D] * C).sum())
    w = (C * C).sum(axis=1)
    cluster = xsq - 2.0 * cross + float((cnt * w).sum())
    total = ALPHA * recon + BETA * cluster
    return (np.float32(total), np.float32(recon), np.float32(cluster))


_nc = None


def _get_nc():
    global _nc
    if _nc is None:
        _nc = build_nc()
    return _nc


def kernel(recon_x, x, cluster_assignments, cluster_centers):
    nc = _get_nc()
    in_maps = host_prepare(recon_x, x, cluster_assignments)
    res = run_bass_kernel_spmd(nc, in_maps, list(range(N_CORES)))
    return host_combine(res.results, cluster_centers)



# revision 2
# speedup vs baseline: 1.2475x; 1.2475x over previous
"""DeepClusterLoss on 8 Trainium2 NeuronCores (Bass/Tile).

reference:
    recon_loss   = sum((recon_x - x)**2)
    cluster_loss = sum((x - centers[assign])**2)
    total        = recon_loss + cluster_loss          (ALPHA = BETA = 1)

Decomposition:
    cluster_loss = sum|x|^2 - 2*sum_k <S_k, C_k> + sum_k n_k*|C_k|^2
where S_k is the per-cluster segment sum of x and n_k the counts.

Device strategy (data-parallel over N):
  - Host sorts each core's samples by cluster id and pads every cluster to
    J*128 rows (J=11 covers the actual max count 1358 with margin).  With
    that layout the segment sums need NO one-hot and NO gather: each
    128-sample slot belongs to exactly one cluster, so a single matmul
    per slot against a constant ones-vector
        out[:, g] += comb_slot[128, 128].T @ ones[128, 1]
    accumulates per-cluster sums in PSUM.  The stationary operand
    comb_slot = [x_slot (64 cols) | x^2_slot (64 cols)] so the same matmul
    also reduces x^2 (rows 64:128 of the PSUM give per-cluster sum of
    squares; only the total is used).
  - Streams are bf16 (exact-to-tolerance for ~1e8-magnitude sums).
  - DVE computes d = r - x and x^2 (2x bf16 mode); ACT squares d with a
    fused per-partition accumulation (recon partials).
  - Host combines the tiny per-core outputs in float64; counts n_k come
    from the host-side bincount that the sort already required.
"""

import sys
from contextlib import ExitStack

import numpy as np

for _p in ("/opt/trn_rl_repo", "/opt/pypackages"):
    if _p not in sys.path:
        sys.path.append(_p)

import ml_dtypes
import concourse.tile as tile
from concourse import bacc, mybir
from concourse.bass_utils import run_bass_kernel_spmd

N, D, K = 1_000_000, 64, 100
ALPHA, BETA = 1.0, 1.0
N_CORES = 8
N_PER_CORE = N // N_CORES   # 125000
P = 128                     # SBUF partitions = samples per slot
J = 11                      # slots per cluster (capacity 1408 >= max 1358)
SLOTS_TOTAL = K * J         # 1100 slots per core
PADDED = SLOTS_TOTAL * P    # 140800 rows per core
SLOTS = 55                  # slots per tile (multiple of J: 5 clusters/tile)
NTILES = SLOTS_TOTAL // SLOTS  # 20
FREE = SLOTS * D            # 3520 bf16 per partition per stream half

_bf16 = mybir.dt.bfloat16
_f32 = mybir.dt.float32
BF16 = ml_dtypes.bfloat16


def build_nc():
    """Build + compile the per-core Bass program (same program on all cores)."""
    nc = bacc.Bacc()
    # xr[t, p, 0, j, :] = x of sample (t*SLOTS+j)*128+p ; xr[t, p, 1, j, :] = r
    xr_d = nc.dram_tensor("xr", [NTILES, P, 2, SLOTS, D], _bf16, kind="ExternalInput")
    # s_out rows 0:64 = per-cluster sum of x (by dim), rows 64:128 = sum of x^2
    s_out = nc.dram_tensor("s_out", [P, K], _f32, kind="ExternalOutput")
    part_out = nc.dram_tensor("partials", [P, NTILES], _f32, kind="ExternalOutput")

    with ExitStack() as ctx:
        tc = ctx.enter_context(tile.TileContext(nc))
        const_pool = ctx.enter_context(tc.tile_pool(name="const", bufs=1))
        xin = ctx.enter_context(tc.tile_pool(name="xin", bufs=3))
        combp = ctx.enter_context(tc.tile_pool(name="comb", bufs=2))
        dp = ctx.enter_context(tc.tile_pool(name="dp", bufs=2))
        sqp = ctx.enter_context(tc.tile_pool(name="sqp", bufs=2))
        psum = ctx.enter_context(tc.tile_pool(name="psum", bufs=1, space="PSUM"))

        ones1 = const_pool.tile([P, 1], _bf16)
        nc.vector.memset(ones1[:], 1.0)
        partials_sb = const_pool.tile([P, NTILES], _f32)

        ps = psum.tile([P, K], _f32)

        for t in range(NTILES):
            xr_t = xin.tile([P, 2, SLOTS, D], _bf16)
            nc.sync.dma_start(xr_t[:], xr_d[t, :, :, :, :])
            x_t = xr_t[:, 0]   # [P, SLOTS, D] contiguous
            r_t = xr_t[:, 1]

            comb_t = combp.tile([P, SLOTS, 2 * D], _bf16)
            nc.vector.tensor_copy(comb_t[:, :, 0:D], x_t)
            nc.vector.tensor_mul(comb_t[:, :, D : 2 * D], x_t, x_t)

            d_t = dp.tile([P, SLOTS, D], _bf16, tag="d")
            nc.vector.tensor_sub(d_t[:], r_t, x_t)
            sq_t = sqp.tile([P, SLOTS, D], _bf16, tag="sq")
            nc.scalar.activation(
                sq_t[:], d_t[:], mybir.ActivationFunctionType.Square,
                accum_out=partials_sb[:, t : t + 1],
            )

            for j in range(SLOTS):
                s = t * SLOTS + j
                g = s // J
                nc.tensor.matmul(
                    ps[:, g : g + 1],
                    comb_t[:, j, :],
                    ones1[:],
                    start=(s % J == 0),
                    stop=(s % J == J - 1),
                )

        s_sb = const_pool.tile([P, K], _f32)
        nc.vector.tensor_copy(s_sb[:], ps[:])
        nc.sync.dma_start(s_out[:, :], s_sb[:])
        nc.sync.dma_start(part_out[:, :], partials_sb[:])

    nc.compile()
    return nc


def host_prepare(recon_x, x, cluster_assignments):
    """Shard, cluster-sort, pad, cast, and lay out the inputs per core."""
    x_np = np.asarray(x, dtype=np.float32).reshape(N_CORES, N_PER_CORE, D)
    r_np = np.asarray(recon_x, dtype=np.float32).reshape(N_CORES, N_PER_CORE, D)
    a_np = np.asarray(cluster_assignments).reshape(N_CORES, N_PER_CORE)
    a_np = a_np.astype(np.int64)

    in_maps = []
    counts = np.zeros((N_CORES, K), np.int64)
    for c in range(N_CORES):
        a = a_np[c]
        cnt = np.bincount(a, minlength=K)
        counts[c] = cnt
        assert cnt.max() <= J * P, f"cluster overflow: {cnt.max()} > {J * P}"
        starts = np.zeros(K, np.int64)
        starts[1:] = np.cumsum(cnt)[:-1]
        order = np.argsort(a, kind="stable")
        g_sorted = a[order]
        dest = g_sorted * (J * P) + (np.arange(N_PER_CORE) - starts[g_sorted])

        xp = np.zeros((PADDED, D), BF16)
        rp = np.zeros((PADDED, D), BF16)
        xp[dest] = x_np[c][order].astype(BF16)
        rp[dest] = r_np[c][order].astype(BF16)

        xr = np.empty((NTILES, P, 2, SLOTS, D), BF16)
        xr[:, :, 0] = xp.reshape(NTILES, SLOTS, P, D).transpose(0, 2, 1, 3)
        xr[:, :, 1] = rp.reshape(NTILES, SLOTS, P, D).transpose(0, 2, 1, 3)
        in_maps.append({"xr": xr})
    return in_maps, counts


def host_combine(results, counts, cluster_centers):
    """Reduce per-core outputs into (total, recon, cluster) in float64."""
    S = np.zeros((K, D), np.float64)
    x2 = 0.0
    recon = 0.0
    for rd in results:
        so = rd["s_out"].astype(np.float64)
        S += so[0:D, :].T
        x2 += so[D : 2 * D, :].sum()
        recon += rd["partials"].astype(np.float64).sum()
    C = np.asarray(cluster_centers, dtype=np.float64)
    cross = float((S * C).sum())
    n_k = counts.sum(axis=0).astype(np.float64)
    w = float((n_k * (C * C).sum(axis=1)).sum())
    cluster = x2 - 2.0 * cross + w
    total = ALPHA * recon + BETA * cluster
    return (np.float32(total), np.float32(recon), np.float32(cluster))


_nc = None


def _get_nc():
    global _nc
    if _nc is None:
        _nc = build_nc()
    return _nc


def kernel(recon_x, x, cluster_assignments, cluster_centers):
    nc = _get_nc()
    in_maps, counts = host_prepare(recon_x, x, cluster_assignments)
    res = run_bass_kernel_spmd(nc, in_maps, list(range(N_CORES)))
    return host_combine(res.results, counts, cluster_centers)


# revision 15
# speedup vs baseline: 1.2542x; 1.0054x over previous
"""DeepClusterLoss on 8 Trainium2 NeuronCores (Bass/Tile).

reference:
    recon_loss   = sum((recon_x - x)**2)
    cluster_loss = sum((x - centers[assign])**2)
    total        = recon_loss + cluster_loss          (ALPHA = BETA = 1)

Decomposition:
    cluster_loss = sum|x|^2 - 2*sum_k <S_k, C_k> + sum_k n_k*|C_k|^2
where S_k is the per-cluster segment sum of x and n_k the counts (host-side
bincount, which the cluster-sort requires anyway).

Device strategy (data-parallel over N):
  - Host sorts each core's samples by cluster id and pads every cluster to
    J*128 rows (J=12 even; capacity 1536 >= actual max count 1358).  With
    that layout the segment sums need NO one-hot and NO gather: every PAIR
    of 128-sample slots belongs to one cluster, so a single self-loading
    matmul per pair against a constant ones-vector
        ps[:, g] += xr[128, 2*64].T @ ones[128, 1]
    accumulates S_g directly in PSUM (rows 0:64 = even slot dims, rows
    64:128 = odd slot dims; host adds the halves).  The 128-column bf16
    stationary takes the fast-weight-load path (~35 ns/matmul measured).
  - Streams are bf16 (exact-to-tolerance for ~1e8-magnitude sums).
  - DVE computes d = r - x (2x bf16) and sum(x^2) via the fused
    tensor_tensor_reduce; ACT squares d with fused per-partition accum.
  - Host combines the tiny per-core outputs in float64.
"""

import sys
from contextlib import ExitStack

import numpy as np

for _p in ("/opt/trn_rl_repo", "/opt/pypackages"):
    if _p not in sys.path:
        sys.path.append(_p)

import ml_dtypes
import concourse.tile as tile
from concourse import bacc, mybir
from concourse.bass_utils import run_bass_kernel_spmd

N, D, K = 1_000_000, 64, 100
ALPHA, BETA = 1.0, 1.0
N_CORES = 8
N_PER_CORE = N // N_CORES   # 125000
P = 128                     # SBUF partitions = samples per slot
J = 12                      # slots per cluster (even; capacity 1536 >= 1358)
SLOTS_TOTAL = K * J         # 1200 slots per core
PADDED = SLOTS_TOTAL * P    # 153600 rows per core
SLOTS = 48                  # slots per tile (4 clusters per tile)
NTILES = SLOTS_TOTAL // SLOTS  # 25
PAIRS = SLOTS // 2          # 24 slot-pairs per tile
FREE = SLOTS * D            # 3072 bf16 per partition per stream half

_bf16 = mybir.dt.bfloat16
_f32 = mybir.dt.float32
BF16 = ml_dtypes.bfloat16


def build_nc():
    """Build + compile the per-core Bass program (same program on all cores)."""
    nc = bacc.Bacc()
    # xr[t, p, 0, j, :] = x of sample (t*SLOTS+j)*128+p ; xr[t, p, 1, j, :] = r
    xr_d = nc.dram_tensor("xr", [NTILES, P, 2, SLOTS, D], _bf16, kind="ExternalInput")
    # s_out[0:64, k] + s_out[64:128, k] = S_k
    s_out = nc.dram_tensor("s_out", [P, K], _f32, kind="ExternalOutput")
    # s2_out: per-cluster sums of x^2 (both slot halves); only the total is used
    s2_out = nc.dram_tensor("s2_out", [P, K], _f32, kind="ExternalOutput")
    # partials col t: sum of (r-x)^2
    part_out = nc.dram_tensor("partials", [P, NTILES], _f32, kind="ExternalOutput")

    with ExitStack() as ctx:
        tc = ctx.enter_context(tile.TileContext(nc))
        const_pool = ctx.enter_context(tc.tile_pool(name="const", bufs=1))
        xin = ctx.enter_context(tc.tile_pool(name="xin", bufs=3))
        dp = ctx.enter_context(tc.tile_pool(name="dp", bufs=2))
        sqp = ctx.enter_context(tc.tile_pool(name="sqp", bufs=2))
        x2p = ctx.enter_context(tc.tile_pool(name="x2p", bufs=2))
        psum = ctx.enter_context(tc.tile_pool(name="psum", bufs=1, space="PSUM"))

        ones1 = const_pool.tile([P, 1], _bf16)
        nc.vector.memset(ones1[:], 1.0)
        partials_sb = const_pool.tile([P, NTILES], _f32)

        ps = psum.tile([P, K], _f32, tag="ps")
        ps2 = psum.tile([P, K], _f32, tag="ps2")

        for t in range(NTILES):
            xr_t = xin.tile([P, 2, FREE], _bf16)
            nc.sync.dma_start(xr_t[:], xr_d[t, :, :, :, :])
            x_t = xr_t[:, 0]   # [P, FREE] contiguous
            r_t = xr_t[:, 1]

            # x^2 elementwise; reduced per-cluster by the PE pair-matmuls below
            x2_t = x2p.tile([P, FREE], _bf16, tag="x2")
            nc.vector.tensor_mul(x2_t[:], x_t, x_t)

            d_t = dp.tile([P, FREE], _bf16, tag="d")
            nc.vector.tensor_sub(d_t[:], r_t, x_t)
            sq_t = sqp.tile([P, FREE], _bf16, tag="sq")
            nc.scalar.activation(
                sq_t[:], d_t[:], mybir.ActivationFunctionType.Square,
                accum_out=partials_sb[:, t : t + 1],
            )

            for k in range(PAIRS):
                pp = t * PAIRS + k          # global pair index
                g = pp // (J // 2)          # cluster of this pair
                first = pp % (J // 2) == 0
                last = pp % (J // 2) == J // 2 - 1
                nc.tensor.matmul(
                    ps[:, g : g + 1],
                    xr_t[:, 0, 2 * D * k : 2 * D * (k + 1)],
                    ones1[:],
                    start=first, stop=last,
                )
                nc.tensor.matmul(
                    ps2[:, g : g + 1],
                    x2_t[:, 2 * D * k : 2 * D * (k + 1)],
                    ones1[:],
                    start=first, stop=last,
                )

        s_sb = const_pool.tile([P, K], _f32, tag="s_sb")
        nc.vector.tensor_copy(s_sb[:], ps[:])
        nc.sync.dma_start(s_out[:, :], s_sb[:])
        s2_sb = const_pool.tile([P, K], _f32, tag="s2_sb")
        nc.vector.tensor_copy(s2_sb[:], ps2[:])
        nc.sync.dma_start(s2_out[:, :], s2_sb[:])
        nc.sync.dma_start(part_out[:, :], partials_sb[:])

    nc.compile()
    return nc


def host_prepare(recon_x, x, cluster_assignments):
    """Shard, cluster-sort, pad, cast, and lay out the inputs per core."""
    x_np = np.asarray(x, dtype=np.float32).reshape(N_CORES, N_PER_CORE, D)
    r_np = np.asarray(recon_x, dtype=np.float32).reshape(N_CORES, N_PER_CORE, D)
    a_np = np.asarray(cluster_assignments).reshape(N_CORES, N_PER_CORE)
    a_np = a_np.astype(np.int64)

    in_maps = []
    counts = np.zeros((N_CORES, K), np.int64)
    for c in range(N_CORES):
        a = a_np[c]
        cnt = np.bincount(a, minlength=K)
        counts[c] = cnt
        assert cnt.max() <= J * P, f"cluster overflow: {cnt.max()} > {J * P}"
        starts = np.zeros(K, np.int64)
        starts[1:] = np.cumsum(cnt)[:-1]
        order = np.argsort(a, kind="stable")
        g_sorted = a[order]
        dest = g_sorted * (J * P) + (np.arange(N_PER_CORE) - starts[g_sorted])

        xp = np.zeros((PADDED, D), BF16)
        rp = np.zeros((PADDED, D), BF16)
        xp[dest] = x_np[c][order].astype(BF16)
        rp[dest] = r_np[c][order].astype(BF16)

        xr = np.empty((NTILES, P, 2, SLOTS, D), BF16)
        xr[:, :, 0] = xp.reshape(NTILES, SLOTS, P, D).transpose(0, 2, 1, 3)
        xr[:, :, 1] = rp.reshape(NTILES, SLOTS, P, D).transpose(0, 2, 1, 3)
        in_maps.append({"xr": xr})
    return in_maps, counts


def host_combine(results, counts, cluster_centers):
    """Reduce per-core outputs into (total, recon, cluster) in float64."""
    S = np.zeros((K, D), np.float64)
    x2 = 0.0
    recon = 0.0
    for rd in results:
        so = rd["s_out"].astype(np.float64)
        S += (so[0:D, :] + so[D : 2 * D, :]).T
        x2 += rd["s2_out"].astype(np.float64).sum()
        recon += rd["partials"].astype(np.float64).sum()
    C = np.asarray(cluster_centers, dtype=np.float64)
    cross = float((S * C).sum())
    n_k = counts.sum(axis=0).astype(np.float64)
    w = float((n_k * (C * C).sum(axis=1)).sum())
    cluster = x2 - 2.0 * cross + w
    total = ALPHA * recon + BETA * cluster
    return (np.float32(total), np.float32(recon), np.float32(cluster))


_nc = None


def _get_nc():
    global _nc
    if _nc is None:
        _nc = build_nc()
    return _nc


def kernel(recon_x, x, cluster_assignments, cluster_centers):
    nc = _get_nc()
    in_maps, counts = host_prepare(recon_x, x, cluster_assignments)
    res = run_bass_kernel_spmd(nc, in_maps, list(range(N_CORES)))
    return host_combine(res.results, counts, cluster_centers)
